# revision 1
# baseline (speedup 1.0000x reference)
"""Laplacian normalization kernel for Trainium2 (8 NeuronCores, SPMD).

out = D^-1/2 A D^-1/2 where D = diag(row sums of A), A: [8192, 8192] fp32.

Sharding: rows split across 8 cores (1024 rows each). Per core:
  pass 1: stream stripes 0-3 first (quarter-width units), then load
    stripes 4-7 into RESIDENT SBUF tiles (16MB cache). Row sums reduce
    per unit; isq = 1/sqrt(deg) is finished per stripe.
  TWO AllGathers: AG1 ships stripes 0-3's isq chunks while stripes 4-7
    are still loading, AG2 ships the rest. AG1's output covers every
    output column j with (j mod 1024) < 512, so half of the scaling and
    stores run during the window where the kernel used to idle waiting
    on a single collective (which is bound by the slowest core).
  pass 2: out = (A * r[:,None]) * c[None,:], one fused DVE op per
    (unit, collective-half), strided over the covered column ranges.

Ring discipline: pass-2 reloads ride the sync HWDGE ring and stores ride
the scalar ring exclusively, so a store blocked on a post-collective
multiply can never sit ahead of an eligible reload in ring FIFO order.
Tiny latency-critical DMAs (isq writes, broadcasts) go via SWDGE.
"""

import sys

sys.path.insert(0, "/opt/trn_rl_repo")

import numpy as np

import concourse.bacc as bacc
import concourse.tile as tile
from concourse import mybir
from concourse.bass_utils import run_bass_kernel_spmd

N = 8192          # full matrix dim
CORES = 8
R = N // CORES    # rows per core: 1024
P = 128           # partitions
S = R // P        # row stripes per core: 8
HW = 4096         # resident half width
QW = 2048         # stream quarter width
NRES = 4          # stripes 4-7 resident in SBUF
HAG = R // 2      # isq elements per collective half: 512
F32 = mybir.dt.float32
MUL = mybir.AluOpType.mult
X = mybir.AxisListType.X

_CACHE = {}


def build_nc():
    if "nc" in _CACHE:
        return _CACHE["nc"]
    nc = bacc.Bacc(
        "TRN2", target_bir_lowering=False, debug=False, num_devices=CORES
    )
    a = nc.dram_tensor("a_block", [R, N], F32, kind="ExternalInput").ap()
    out = nc.dram_tensor("out_block", [R, N], F32, kind="ExternalOutput").ap()

    with tile.TileContext(nc) as tc:
        with (
            tc.tile_pool(name="dram", bufs=1, space="DRAM") as dram,
            tc.tile_pool(name="res", bufs=1) as res,
            tc.tile_pool(name="stream", bufs=4) as stream,
            tc.tile_pool(name="cpool", bufs=1) as cpool,
            tc.tile_pool(name="small", bufs=1) as small,
        ):
            # separate DRAM tensors per collective half so AG1's input
            # dependency can never couple to stripes 4-7's writes
            isq_loc = [
                dram.tile([HAG], F32, name=f"isq_loc{g}") for g in range(2)
            ]
            isq_ag = [
                dram.tile(
                    [CORES * HAG], F32, addr_space="Shared", name=f"isq_ag{g}"
                )
                for g in range(2)
            ]

            part = small.tile([P, 4 * S], F32)   # partial row sums
            isq_sb = small.tile([P, S], F32)     # per-stripe row scale

            def finish_stripe(s, nparts):
                """Combine partials -> isq -> isq_sb + DRAM chunk."""
                for i in range(1, nparts):
                    nc.vector.tensor_add(
                        part[:, 4 * s : 4 * s + 1],
                        part[:, 4 * s : 4 * s + 1],
                        part[:, 4 * s + i : 4 * s + i + 1],
                    )
                nc.vector.reciprocal(
                    part[:, 4 * s : 4 * s + 1], part[:, 4 * s : 4 * s + 1]
                )
                nc.scalar.sqrt(
                    isq_sb[:, s : s + 1], part[:, 4 * s : 4 * s + 1]
                )
                g, off = divmod(s * P, HAG)
                nc.gpsimd.dma_start(
                    isq_loc[g][off : off + P].unsqueeze(1),
                    isq_sb[:, s : s + 1],
                )

            # ---- pass 1 ----
            # streamed stripes 0-3 first: their isq feeds AG1, and their
            # reduces free the stream slots for pass-2 reloads early
            nunit = 0
            for s in range(S - NRES):
                for q in range(N // QW):
                    t = stream.tile([P, QW], F32, tag="stream")
                    ld = nc.sync if nunit % 2 == 0 else nc.scalar
                    ld.dma_start(
                        t[:], a[s * P : (s + 1) * P, q * QW : (q + 1) * QW]
                    )
                    nc.vector.reduce_sum(
                        out=part[:, 4 * s + q : 4 * s + q + 1], in_=t[:], axis=X
                    )
                    nunit += 1
                finish_stripe(s, N // QW)

            ag_args = dict(
                replica_groups=[list(range(CORES))],
            )
            nc.gpsimd.collective_compute(
                "AllGather",
                mybir.AluOpType.bypass,
                ins=[isq_loc[0][:].opt()],
                outs=[isq_ag[0][:].opt()],
                **ag_args,
            )

            # resident stripes 4-7, kept for pass 2
            res_tiles = {}
            for s in range(S - NRES, S):
                for h in range(N // HW):
                    t = res.tile([P, HW], F32, tag=f"res{s}_{h}", bufs=1)
                    ld = nc.sync if nunit % 2 == 0 else nc.scalar
                    ld.dma_start(
                        t[:], a[s * P : (s + 1) * P, h * HW : (h + 1) * HW]
                    )
                    nc.vector.reduce_sum(
                        out=part[:, 4 * s + h : 4 * s + h + 1], in_=t[:], axis=X
                    )
                    res_tiles[(s, h)] = t
                    nunit += 1
                finish_stripe(s, N // HW)

            nc.gpsimd.collective_compute(
                "AllGather",
                mybir.AluOpType.bypass,
                ins=[isq_loc[1][:].opt()],
                outs=[isq_ag[1][:].opt()],
                **ag_args,
            )

            # column-scale broadcast. AG half g covers, within each 1024
            # column block, columns [g*512, g*512+512). isq_ag[g] is
            # ordered (core, stripe-offset): element k*512 + u = isq of
            # global row k*1024 + g*512 + u = scale for that column.
            # cb[g][h] holds half g's scales for output columns
            # [h*4096, (h+1)*4096), packed compactly ([m*512+u] layout):
            # one tile per AG half, so the early multiplies can never
            # pick up a false dependency on the late collective.
            cb = [
                [
                    cpool.tile(
                        [P, HW // 2],
                        F32,
                        tag=f"cb{g}{h}",
                        bufs=1,
                        name=f"cb{g}{h}",
                    )
                    for h in range(N // HW)
                ]
                for g in range(2)
            ]
            for g in range(2):
                for h in range(N // HW):
                    src = (
                        isq_ag[g][h * (HW // 2) : (h + 1) * (HW // 2)]
                        .rearrange("(m c) -> m c", c=HAG)
                        .unsqueeze(0)
                        .to_broadcast([P, HW // 1024, HAG])
                    )
                    nc.gpsimd.dma_start(
                        cb[g][h][:].rearrange("p (m c) -> p m c", c=HAG), src
                    )

            # ---- pass 2: out = (A * r) * c ----
            def scale_store(s, col0, t, width, g):
                """Scale + store the AG-half-g columns of tile t."""
                h, hoff = divmod(col0, HW)
                m0 = hoff // 1024
                m = width // 1024
                c_ap = cb[g][h][
                    :, m0 * HAG : (m0 + m) * HAG
                ].rearrange("p (m c) -> p m c", c=HAG)
                nc.vector.scalar_tensor_tensor(
                    out=c3(t[:], 0, width, g),
                    in0=c3(t[:], 0, width, g),
                    scalar=isq_sb[:, s : s + 1],
                    in1=c_ap,
                    op0=MUL,
                    op1=MUL,
                )
                nc.scalar.dma_start(
                    c3(out[s * P : (s + 1) * P, :], col0, width, g),
                    c3(t[:], 0, width, g),
                )

            # resident stripes: AG1-covered columns first (those flow
            # while AG2 is still waiting on the slowest core)
            for s in range(S - NRES, S):
                for h in range(N // HW):
                    scale_store(s, h * HW, res_tiles[(s, h)], HW, 0)
            for s in range(S - NRES, S):
                for h in range(N // HW):
                    scale_store(s, h * HW, res_tiles[(s, h)], HW, 1)

            # streamed stripes reload on the sync ring, quarter width
            for s in range(S - NRES):
                for q in range(N // QW):
                    t = stream.tile([P, QW], F32, tag="stream")
                    nc.sync.dma_start(
                        t[:], a[s * P : (s + 1) * P, q * QW : (q + 1) * QW]
                    )
                    scale_store(s, q * QW, t, QW, 0)
                    scale_store(s, q * QW, t, QW, 1)

    nc.compile()
    _CACHE["nc"] = nc
    return nc


def c3(ap, col0, width, g):
    """The AG-half-g columns of ap's column range [col0, col0+width):
    within each 1024-column block, columns [g*512, g*512+512), as a
    strided [P, width//1024, 512] access pattern."""
    return ap[:, col0 : col0 + width].rearrange("p (m c) -> p m c", c=1024)[
        :, :, g * HAG : (g + 1) * HAG
    ]


def kernel(adjacency_matrix):
    A = np.ascontiguousarray(np.asarray(adjacency_matrix, dtype=np.float32))
    assert A.shape == (N, N)
    nc = build_nc()
    in_maps = [
        {"a_block": np.ascontiguousarray(A[k * R : (k + 1) * R])}
        for k in range(CORES)
    ]
    res = run_bass_kernel_spmd(nc, in_maps, list(range(CORES)))
    return np.concatenate(
        [res.results[k]["out_block"] for k in range(CORES)], axis=0
    )



# revision 2
# speedup vs baseline: 1.1665x; 1.1665x over previous
"""Laplacian normalization kernel for Trainium2 (8 NeuronCores, SPMD).

out = D^-1/2 A D^-1/2 where D = diag(row sums of A), A: [8192, 8192] fp32.

Sharding: rows split across 8 cores (1024 rows each, 8 stripes of 128).

Single-read design (64 MB/core HBM traffic = the roofline floor):
  pass A: stream each stripe once as two [128, 4096] f32 half-tiles on
    the sync HWDGE ring. One ACT `activation(Copy)` per half both
    downcasts into a RESIDENT bf16 stripe (16 MB total SBUF) and emits
    the row-sum via accum_out - no separate reduce pass, and no pass-B
    reload from HBM.
  FOUR AllGathers, one per stripe pair: AG_q ships isq for local rows
    [q*256,(q+1)*256), i.e. output columns with (j mod 1024) in that
    range. Quarter-granular unlock keeps >=8 MB of store work eligible
    whenever a later collective is still waiting on a straggler core,
    so inter-core launch skew hides behind stores instead of idling DMA.
  pass B: out = (bf16A * r) * c, one fused scalar_tensor_tensor per
    (stripe, quarter) into an f32 staging tile, stores alternating
    between the scalar and sync HWDGE rings (both free after pass A).

Column scales live in bf16 broadcast tiles (cast+replicate SWDGE DMA
from the gathered vector). Total rounding error ~2*2^-9 << 2e-2 gate.
"""

import sys

sys.path.insert(0, "/opt/trn_rl_repo")

import numpy as np

import concourse.bacc as bacc
import concourse.tile as tile
from concourse import mybir
from concourse.bass_utils import run_bass_kernel_spmd

N = 8192          # full matrix dim
CORES = 8
R = N // CORES    # rows per core: 1024
P = 128           # partitions
S = R // P        # row stripes per core: 8
HW = N // 2       # half-stripe load width: 4096
NQ = 4            # collective quarters (stripe pairs)
QAG = R // NQ     # isq elements per quarter: 256
F32 = mybir.dt.float32
BF16 = mybir.dt.bfloat16
MUL = mybir.AluOpType.mult
COPY = mybir.ActivationFunctionType.Copy

_CACHE = {}


def build_nc():
    if "nc" in _CACHE:
        return _CACHE["nc"]
    nc = bacc.Bacc(
        "TRN2", target_bir_lowering=False, debug=False, num_devices=CORES
    )
    a = nc.dram_tensor("a_block", [R, N], F32, kind="ExternalInput").ap()
    out = nc.dram_tensor("out_block", [R, N], F32, kind="ExternalOutput").ap()

    with tile.TileContext(nc) as tc:
        with (
            tc.tile_pool(name="dram", bufs=1, space="DRAM") as dram,
            tc.tile_pool(name="res", bufs=1) as res,
            tc.tile_pool(name="work", bufs=3) as work,
            tc.tile_pool(name="cpool", bufs=1) as cpool,
            tc.tile_pool(name="small", bufs=1) as small,
        ):
            isq_loc = [
                dram.tile([QAG], F32, name=f"isq_loc{q}") for q in range(NQ)
            ]
            isq_ag = [
                dram.tile(
                    [CORES * QAG], F32, addr_space="Shared", name=f"isq_ag{q}"
                )
                for q in range(NQ)
            ]

            part = small.tile([P, 2 * S], F32)   # half-row sums
            isq_sb = small.tile([P, S], F32)     # per-stripe row scale

            res_tiles = [
                res.tile([P, N], BF16, tag=f"res{s}", bufs=1, name=f"res{s}")
                for s in range(S)
            ]
            # col-scale broadcast tiles: cb[q][p, m*QAG + u] = isq of
            # global row m*1024 + q*QAG + u = scale for that column
            cb = [
                cpool.tile([P, CORES * QAG], BF16, tag=f"cb{q}", bufs=1,
                           name=f"cb{q}")
                for q in range(NQ)
            ]

            ag_args = dict(replica_groups=[list(range(CORES))])

            # ---- pass A: single streamed read, fused cast+rowsum ----
            for s in range(S):
                for h in range(2):
                    t = work.tile([P, HW], F32, tag="work")
                    nc.sync.dma_start(
                        t[:], a[s * P : (s + 1) * P, h * HW : (h + 1) * HW]
                    )
                    nc.scalar.activation(
                        out=res_tiles[s][:, h * HW : (h + 1) * HW],
                        in_=t[:],
                        func=COPY,
                        accum_out=part[:, 2 * s + h : 2 * s + h + 1],
                    )
                nc.vector.tensor_add(
                    part[:, 2 * s : 2 * s + 1],
                    part[:, 2 * s : 2 * s + 1],
                    part[:, 2 * s + 1 : 2 * s + 2],
                )
                nc.vector.reciprocal(
                    part[:, 2 * s : 2 * s + 1], part[:, 2 * s : 2 * s + 1]
                )
                nc.scalar.sqrt(
                    isq_sb[:, s : s + 1], part[:, 2 * s : 2 * s + 1]
                )
                q, off = divmod(s * P, QAG)
                nc.gpsimd.dma_start(
                    isq_loc[q][off : off + P].unsqueeze(1),
                    isq_sb[:, s : s + 1],
                )
                if s % 2 == 1:
                    nc.gpsimd.collective_compute(
                        "AllGather",
                        mybir.AluOpType.bypass,
                        ins=[isq_loc[q][:].opt()],
                        outs=[isq_ag[q][:].opt()],
                        **ag_args,
                    )
                    # cast+replicate the gathered quarter across the
                    # 128 partitions (SWDGE: broadcast AP + f32->bf16)
                    nc.gpsimd.dma_start(
                        cb[q][:],
                        isq_ag[q][:].unsqueeze(0).to_broadcast(
                            [P, CORES * QAG]
                        ),
                    )

            # ---- pass B: out = (A * r) * c ----
            def q3(ap, q):
                """Quarter-q columns of [P, N] ap: within each 1024
                block, columns [q*QAG, (q+1)*QAG) -> [P, 8, QAG]."""
                return ap.rearrange("p (m c) -> p m c", c=R)[
                    :, :, q * QAG : (q + 1) * QAG
                ]

            for q in range(NQ):
                for s in range(S):
                    o = work.tile([P, CORES * QAG], F32, tag="work")
                    nc.vector.scalar_tensor_tensor(
                        out=o[:].rearrange("p (m c) -> p m c", c=QAG),
                        in0=q3(res_tiles[s][:], q),
                        scalar=isq_sb[:, s : s + 1],
                        in1=cb[q][:].rearrange("p (m c) -> p m c", c=QAG),
                        op0=MUL,
                        op1=MUL,
                    )
                    st = nc.sync if s % 2 == 1 else nc.scalar
                    st.dma_start(
                        q3(out[s * P : (s + 1) * P, :], q),
                        o[:].rearrange("p (m c) -> p m c", c=QAG),
                    )

    nc.compile()
    _CACHE["nc"] = nc
    return nc


def kernel(adjacency_matrix):
    A = np.ascontiguousarray(np.asarray(adjacency_matrix, dtype=np.float32))
    assert A.shape == (N, N)
    nc = build_nc()
    in_maps = [
        {"a_block": np.ascontiguousarray(A[k * R : (k + 1) * R])}
        for k in range(CORES)
    ]
    res = run_bass_kernel_spmd(nc, in_maps, list(range(CORES)))
    return np.concatenate(
        [res.results[k]["out_block"] for k in range(CORES)], axis=0
    )


# revision 3
# speedup vs baseline: 1.2484x; 1.0702x over previous
"""Laplacian normalization kernel for Trainium2 (8 NeuronCores, SPMD).

out = D^-1/2 A D^-1/2 where D = diag(row sums of A), A: [8192, 8192] fp32.

Sharding: rows split across 8 cores (1024 rows each, 8 stripes of 128).

Single-read design (64 MB/core HBM traffic = the roofline floor):
  pass A: each stripe is ONE fully-contiguous 4 MB SWDGE cast-DMA
    (f32 HBM -> resident bf16 SBUF tile). SWDGE pre-emits all eight
    DMAs' descriptors into its ring, so the read stream is gapless and
    every HBM access is a whole 32 KB row (no half-row interleave).
    Row sums come from DVE reduce over the bf16 tiles (2x rate);
    deg -> isq via DVE reciprocal + ACT sqrt.
  isq chunk writes ride the otherwise-idle sync HWDGE ring so they are
    never head-of-line blocked behind the big SWDGE reads; the Pool
    queue carries only the four AllGather triggers (+ cb fills).
  FOUR AllGathers, one per stripe pair: AG_q ships isq for local rows
    [q*256,(q+1)*256), i.e. output columns with (j mod 1024) in that
    range. Quarter-granular unlock keeps store work eligible while a
    later collective still waits on a straggler core.
  pass B: out = (bf16A * r) * c, one fused scalar_tensor_tensor per
    (stripe, quarter) into an f32 staging tile, stores alternating
    between the scalar and sync HWDGE rings.

Column scales live in bf16 broadcast tiles (cast+replicate SWDGE DMA
from the gathered vector). Total rounding error ~2*2^-9 << 2e-2 gate.
"""

import sys

sys.path.insert(0, "/opt/trn_rl_repo")

import numpy as np

import concourse.bacc as bacc
import concourse.tile as tile
from concourse import mybir
from concourse.bass_utils import run_bass_kernel_spmd

N = 8192          # full matrix dim
CORES = 8
R = N // CORES    # rows per core: 1024
P = 128           # partitions
S = R // P        # row stripes per core: 8
NQ = 4            # collective quarters (stripe pairs)
QAG = R // NQ     # isq elements per quarter: 256
F32 = mybir.dt.float32
BF16 = mybir.dt.bfloat16
MUL = mybir.AluOpType.mult
X = mybir.AxisListType.X

_CACHE = {}


def build_nc():
    if "nc" in _CACHE:
        return _CACHE["nc"]
    nc = bacc.Bacc(
        "TRN2", target_bir_lowering=False, debug=False, num_devices=CORES
    )
    a = nc.dram_tensor("a_block", [R, N], F32, kind="ExternalInput").ap()
    out = nc.dram_tensor("out_block", [R, N], F32, kind="ExternalOutput").ap()

    with tile.TileContext(nc) as tc:
        with (
            tc.tile_pool(name="dram", bufs=1, space="DRAM") as dram,
            tc.tile_pool(name="res", bufs=1) as res,
            tc.tile_pool(name="work", bufs=4) as work,
            tc.tile_pool(name="cpool", bufs=1) as cpool,
            tc.tile_pool(name="small", bufs=1) as small,
        ):
            isq_loc = [
                dram.tile([QAG], F32, name=f"isq_loc{q}") for q in range(NQ)
            ]
            isq_ag = [
                dram.tile(
                    [CORES * QAG], F32, addr_space="Shared", name=f"isq_ag{q}"
                )
                for q in range(NQ)
            ]

            part = small.tile([P, S], F32)       # row sums -> 1/deg
            isq_sb = small.tile([P, S], F32)     # per-stripe row scale

            res_tiles = [
                res.tile([P, N], BF16, tag=f"res{s}", bufs=1, name=f"res{s}")
                for s in range(S)
            ]
            # col-scale broadcast tiles: cb[q][p, m*QAG + u] = isq of
            # global row m*1024 + q*QAG + u = scale for that column
            cb = [
                cpool.tile([P, CORES * QAG], BF16, tag=f"cb{q}", bufs=1,
                           name=f"cb{q}")
                for q in range(NQ)
            ]

            ag_args = dict(replica_groups=[list(range(CORES))])

            # ---- pass A ----
            # all eight cast-reads first in Pool program order: their
            # descriptors hit the SWDGE ring up front, nothing can
            # head-of-line block them, engines drain back to back
            for s in range(S):
                nc.gpsimd.dma_start(res_tiles[s][:], a[s * P : (s + 1) * P, :])

            for s in range(S):
                nc.vector.reduce_sum(
                    out=part[:, s : s + 1], in_=res_tiles[s][:], axis=X
                )
                nc.vector.reciprocal(
                    part[:, s : s + 1], part[:, s : s + 1]
                )
                nc.scalar.sqrt(
                    isq_sb[:, s : s + 1], part[:, s : s + 1]
                )
                q, off = divmod(s * P, QAG)
                nc.sync.dma_start(
                    isq_loc[q][off : off + P].unsqueeze(1),
                    isq_sb[:, s : s + 1],
                )
                if s % 2 == 1:
                    nc.gpsimd.collective_compute(
                        "AllGather",
                        mybir.AluOpType.bypass,
                        ins=[isq_loc[q][:].opt()],
                        outs=[isq_ag[q][:].opt()],
                        **ag_args,
                    )

            # cast+replicate each gathered quarter across partitions
            # (SWDGE; queues behind the big reads, lands right as the
            # read stream drains)
            for q in range(NQ):
                nc.gpsimd.dma_start(
                    cb[q][:],
                    isq_ag[q][:].unsqueeze(0).to_broadcast([P, CORES * QAG]),
                )

            # ---- pass B: out = (A * r) * c ----
            def q3(ap, q):
                """Quarter-q columns of [P, N] ap: within each 1024
                block, columns [q*QAG, (q+1)*QAG) -> [P, 8, QAG]."""
                return ap.rearrange("p (m c) -> p m c", c=R)[
                    :, :, q * QAG : (q + 1) * QAG
                ]

            for q in range(NQ):
                for s in range(S):
                    o = work.tile([P, CORES * QAG], F32, tag="work")
                    nc.vector.scalar_tensor_tensor(
                        out=o[:].rearrange("p (m c) -> p m c", c=QAG),
                        in0=q3(res_tiles[s][:], q),
                        scalar=isq_sb[:, s : s + 1],
                        in1=cb[q][:].rearrange("p (m c) -> p m c", c=QAG),
                        op0=MUL,
                        op1=MUL,
                    )
                    st = nc.sync if s % 2 == 1 else nc.scalar
                    st.dma_start(
                        q3(out[s * P : (s + 1) * P, :], q),
                        o[:].rearrange("p (m c) -> p m c", c=QAG),
                    )

    nc.compile()
    _CACHE["nc"] = nc
    return nc


def kernel(adjacency_matrix):
    A = np.ascontiguousarray(np.asarray(adjacency_matrix, dtype=np.float32))
    assert A.shape == (N, N)
    nc = build_nc()
    in_maps = [
        {"a_block": np.ascontiguousarray(A[k * R : (k + 1) * R])}
        for k in range(CORES)
    ]
    res = run_bass_kernel_spmd(nc, in_maps, list(range(CORES)))
    return np.concatenate(
        [res.results[k]["out_block"] for k in range(CORES)], axis=0
    )
